# revision 49
# baseline (speedup 1.0000x reference)
"""Trainium2 Bass kernel for fused sparse attention (policy-masked softmax).

Computation (per batch b):
    qkv  = x @ qkv_w.T + qkv_b                  -> q, k, v   [H heads, hd=64]
    S    = (q @ k.T) * hd**-0.5                 [H, N, N]
    P    = eps-softmax(S) with key-policy mask and eye-blend
    out  = (P @ v) @ proj_w.T + proj_b

Strategy: pure data-parallel over batch across 8 NeuronCores (4 batches
per core), fully fused on-chip per batch.  Host pre-transposes x and the
weights (and pre-packs them in SBUF partition layout) so the device
kernel needs no transposes:
  - x^T [C, N] tiles are the shared lhsT/rhs for the QKV projections
  - q^T/k^T land as [64, N] head slices (contraction dim on partitions)
  - softmax runs in the S^T [key, query] orientation: the policy mask is a
    per-partition scalar, the attn row-sum rides along the attn@v matmul
    via a per-head all-ones lhsT column in v_ext (written once), and 1/sum
    is applied via reciprocal_approx_fast + gpsimd partition-broadcast.
  - attn output accumulates directly in proj-ready [C, N] layout.
  - batch b+1's q/k projection chains are interleaved into batch b's
    attention loop as independent PE filler work.
Matmul operands are fp16 (1 cycle/row on the PE vs 2 for f32r/4 for fp32,
fp32 PSUM accumulation, ~5e-4 relative error).  Softmax skips the
max-subtraction (scores are O(1) here), which cancels exactly except for
the eps terms (~1e-8 relative).
"""

import sys

if "/opt/trn_rl_repo" not in sys.path:
    sys.path.insert(0, "/opt/trn_rl_repo")

import numpy as np

B, N, C, H = 32, 384, 768, 12
HD = C // H  # 64
NCORES = 8
BL = B // NCORES  # batches per core
EPS = 1e-6
SCALE = HD ** -0.5
P = 128
KT = C // P   # 6 contraction tiles over C
NT = N // P   # 3 tiles over sequence
VS = 128      # per-head v stride in v_ext: [ones | 63 zeros | v(64)]
VOFF = 64     # v offset within a head's block (psum reads from base 64)
JQK = 2 * C // P  # 12 q/k output tiles

_CACHE = {}


def _build_nc():
    import concourse.tile as tile
    from concourse import bacc, mybir

    F32 = mybir.dt.float32
    F16 = mybir.dt.float16
    EXP = mybir.ActivationFunctionType.Exp
    IDENT = mybir.ActivationFunctionType.Identity
    MULT = mybir.AluOpType.mult
    ADD = mybir.AluOpType.add
    NE = mybir.AluOpType.not_equal

    nc = bacc.Bacc(None, target_bir_lowering=False)

    xT_d = nc.declare_dram_parameter("xT", [BL, P, KT, N], F16, isOutput=False)
    pol_d = nc.declare_dram_parameter("pol", [BL, P, NT], F32, isOutput=False)
    wqkT_d = nc.declare_dram_parameter("wqkT", [P, KT, 2 * C], F16, isOutput=False)
    wvT_d = nc.declare_dram_parameter("wvT", [P, KT, C], F16, isOutput=False)
    bqk_d = nc.declare_dram_parameter("bqk", [P, JQK], F32, isOutput=False)
    bv_d = nc.declare_dram_parameter("bv", [C], F32, isOutput=False)
    wpT_d = nc.declare_dram_parameter("wpT", [P, KT, C], F16, isOutput=False)
    bp_d = nc.declare_dram_parameter("bp", [C], F32, isOutput=False)
    out_d = nc.declare_dram_parameter("out", [BL, N, C], F32, isOutput=True)

    import concourse.bass as bass

    def bcast_dram(vec_ap, parts):
        # partition-broadcast a 1-D DRAM vector: step 0 over partitions
        return bass.AP(
            tensor=vec_ap.tensor,
            offset=vec_ap.offset,
            ap=[[0, parts]] + list(vec_ap.ap),
        )

    with tile.TileContext(nc) as tc:
        with (
            tc.tile_pool(name="singles", bufs=1) as singles,
            tc.tile_pool(name="xin", bufs=BL) as xin,
            tc.tile_pool(name="mid", bufs=3) as mid,
            tc.tile_pool(name="eact", bufs=8) as eact,
            tc.tile_pool(name="ehatp", bufs=4) as ehatp,
            tc.tile_pool(name="small", bufs=6) as small,
            tc.tile_pool(name="outp", bufs=2) as outp,
            tc.tile_pool(name="ps3", bufs=6, space="PSUM") as ps3,
            tc.tile_pool(name="ps2", bufs=2, space="PSUM") as ps2,
        ):
            # ---- tiny tensors + first batch's inputs first, so qk(b=0)
            # and its psum evictions can start as early as possible
            bqk_sb = singles.tile([P, JQK], F32)
            nc.sync.dma_start(out=bqk_sb, in_=bqk_d[:, :])
            # dummy exp pulls the one-time ACT table load off the critical path
            warm = singles.tile([1, 1], F32)
            nc.vector.memset(warm, 0.0)
            nc.scalar.activation(out=warm, in_=warm, func=EXP, scale=1.0)
            pol_sbs = [xin.tile([P, NT], F32, tag="pol", name=f"pol{b}")
                       for b in range(BL)]
            nc.sync.dma_start(out=pol_sbs[0], in_=pol_d[0])
            xT_sbs = [xin.tile([P, KT, N], F16, tag="xT", name=f"xT{b}")
                      for b in range(BL)]
            for k0 in range(0, KT, 2):
                nc.sync.dma_start(
                    out=xT_sbs[0][:, k0 : k0 + 2, :], in_=xT_d[0, :, k0 : k0 + 2, :]
                )

            # weights in 384-column tiles so consumers only wait on the
            # columns they actually read
            def load_w(dram, name, col0):
                tiles = []
                for i, j0 in enumerate(range(0, C, 384)):
                    t = singles.tile([P, KT, 384], F16, tag=f"{name}{i}",
                                     name=f"{name}{i}")
                    for k0 in range(0, KT, 2):
                        nc.sync.dma_start(
                            out=t[:, k0 : k0 + 2, :],
                            in_=dram[:, k0 : k0 + 2, col0 + j0 : col0 + j0 + 384],
                        )
                    tiles.append(t)
                return tiles

            wq_t = load_w(wqkT_d, "wq", 0)
            wk_t = load_w(wqkT_d, "wk", C)
            wv_t = load_w(wvT_d, "wv", 0)
            wp_t = load_w(wpT_d, "wp", 0)
            bv_sb = singles.tile([P, C], F32)
            nc.gpsimd.dma_start(out=bv_sb, in_=bcast_dram(bv_d[:], P))
            bp_sb = singles.tile([P, C], F32)
            nc.gpsimd.dma_start(out=bp_sb, in_=bcast_dram(bp_d[:], P))

            # remaining batches' inputs
            for b in range(1, BL):
                nc.sync.dma_start(out=xT_sbs[b], in_=xT_d[b])
                nc.sync.dma_start(out=pol_sbs[b], in_=pol_d[b])

            # ---- persistent v_ext buffers: [ones | 63 zeros | v] per head;
            # ones/zeros written once, only the v(64) regions change per batch
            v_exts = []
            for i in range(2):
                ve = singles.tile([P, NT, H * VS], F16, tag=f"ve{i}")
                nc.vector.memset(ve, 0.0)
                nc.vector.memset(
                    ve.rearrange("p t (h s) -> p t h s", s=VS)[:, :, :, 0:1], 1.0
                )
                v_exts.append(ve)

            # ---- blend tiles: blend[p, t, m] = 1 if m == t*128+p else pol[p]
            blends = []
            for b in range(BL):
                blend = xin.tile([P, NT, N], F16, tag="blend", name=f"bl{b}")
                for t in range(NT):
                    nc.scalar.activation(
                        out=blend[:, t, :], in_=xT_sbs[b][:, 0, :],
                        func=IDENT, bias=pol_sbs[b][:, t : t + 1], scale=0.0,
                    )
                    nc.gpsimd.affine_select(
                        out=blend[:, t, :], in_=blend[:, t, :],
                        compare_op=NE, fill=1.0, base=t * P,
                        pattern=[[-1, N]], channel_multiplier=1,
                    )
                blends.append(blend)

            # ================= per-batch phase emitters =================
            qkTs = {}   # (b, jt) -> tile

            def qk_chain(b, jt):
                t = mid.tile([P, N], F16, tag=f"qkT{jt}", name=f"qk{b}_{jt}")
                qkTs[(b, jt)] = t
                ps = ps3.tile([P, 512], F32, tag="mm")
                half = wq_t if jt < JQK // 2 else wk_t
                joff = (jt % (JQK // 2)) * P
                wtile = half[joff // 384]
                for kt in range(KT):
                    nc.tensor.matmul(
                        ps[:, :N],
                        wtile[:, kt, joff % 384 : joff % 384 + P],
                        xT_sbs[b][:, kt, :],
                        start=(kt == 0), stop=(kt == KT - 1),
                    )
                # bias add + fp16 round (psum -> sbuf), alternating engines
                if jt % 2 == 0:
                    nc.scalar.activation(
                        out=t, in_=ps[:, :N],
                        func=IDENT, bias=bqk_sb[:, jt : jt + 1], scale=1.0,
                    )
                else:
                    nc.vector.tensor_scalar(
                        out=t, in0=ps[:, :N],
                        scalar1=bqk_sb[:, jt : jt + 1], scalar2=None,
                        op0=ADD,
                    )

            def v_chain(b, nt, c0, cw):
                v_ext = v_exts[b % 2]
                v_dst = v_ext.rearrange("p t (h s) -> p t h s", s=VS)
                if True:
                    if True:
                        ps = ps3.tile([P, 512], F32, tag="mm")
                        for kt in range(KT):
                            nc.tensor.matmul(
                                ps[:, :cw],
                                xT_sbs[b][:, kt, nt * P : (nt + 1) * P],
                                wv_t[c0 // 384][:, kt, :],
                                start=(kt == 0), stop=(kt == KT - 1),
                            )
                        h0, hn = c0 // HD, cw // HD
                        nc.vector.tensor_tensor(
                            out=v_dst[:, nt, h0 : h0 + hn, VOFF:VS],
                            in0=ps[:, :cw].rearrange("p (h d) -> p h d", d=HD),
                            in1=bv_sb[:, c0 : c0 + cw].rearrange(
                                "p (h d) -> p h d", d=HD
                            ),
                            op=ADD,
                        )

            def s_phase_pair(b, p):
                # S matmuls for heads (2p, 2p+1) interleaved: consecutive
                # matmuls target PE row groups 0-1 / 2-3 (bases 0 / 64)
                jq, jk = p, JQK // 2 + p
                ps_pair = ([], [])
                for mt in range(NT):
                    for e in range(2):
                        base = e * HD
                        qh = qkTs[(b, jq)][base : base + HD, :]
                        kh = qkTs[(b, jk)][base : base + HD, :]
                        ps_s = ps3.tile([P, 512], F32, tag="mm")
                        nc.tensor.matmul(
                            ps_s[:, :N],
                            kh[:, mt * P : (mt + 1) * P],
                            qh,
                            start=True, stop=True,
                        )
                        ps_pair[e].append(ps_s)
                return ps_pair

            def softmax_av_phase(b, h, ps_ss, oT):
                base = (h % 2) * HD
                jq = h // 2
                v_ext = v_exts[b % 2]
                ehat = [
                    ehatp.tile([P, N], F16, tag=f"ehat{mt}", name=f"eh{mt}")
                    for mt in range(NT)
                ]
                eas = []
                for mt in range(NT):
                    ea = eact.tile([P, N], F16, tag="ea")
                    nc.scalar.activation(
                        out=ea, in_=ps_ss[mt][:, :N], func=EXP, scale=SCALE,
                    )
                    eas.append(ea)
                for mt in range(NT):
                    nc.vector.tensor_tensor(
                        out=ehat[mt], in0=eas[mt], in1=blends[b][:, mt, :],
                        op=MULT,
                    )
                ps_av = ps2.tile([P, 512], F32, tag="av")
                for mt in range(NT):
                    nc.tensor.matmul(
                        ps_av[: VOFF + HD, :N],
                        v_ext[:, mt, h * VS : (h + 1) * VS],
                        ehat[mt],
                        start=(mt == 0), stop=(mt == NT - 1),
                    )
                # r = 1/rowsum (eps negligible vs rowsum >= exp(s_nn)).
                # Sum row at psum partition 0; custom ops need base 0.
                su_sb = small.tile([1, N], F32, tag="su")
                nc.scalar.activation(
                    out=su_sb, in_=ps_av[0:1, :N],
                    func=IDENT, bias=0.0, scale=1.0,
                )
                r_sb = small.tile([1, N], F32, tag="r")
                nc.vector.reciprocal_approx_fast(out=r_sb, in_=su_sb)
                rb_sb = small.tile([HD, N], F32, tag="rb")
                nc.gpsimd.partition_broadcast(rb_sb, r_sb)
                nc.vector.tensor_tensor(
                    out=oT[jq][base : base + HD, :],
                    in0=ps_av[VOFF : VOFF + HD, :N], in1=rb_sb, op=MULT,
                )

            def proj_phase(b, oT):
                out_sb = outp.tile([P, NT, C], F32, tag="out")
                out_v = out_d[b].rearrange("(t p) o -> p t o", p=P)
                for nt in range(NT):
                    for c0, cw in ((0, 384), (384, 384)):
                        ps = ps3.tile([P, 512], F32, tag="mm")
                        for kt in range(KT):
                            nc.tensor.matmul(
                                ps[:, :cw],
                                oT[kt][:, nt * P : (nt + 1) * P],
                                wp_t[c0 // 384][:, kt, :],
                                start=(kt == 0), stop=(kt == KT - 1),
                            )
                        nc.vector.tensor_add(
                            out=out_sb[:, nt, c0 : c0 + cw],
                            in0=ps[:, :cw],
                            in1=bp_sb[:, c0 : c0 + cw],
                        )
                    nc.sync.dma_start(
                        out=out_v[:, nt : nt + 1, :],
                        in_=out_sb[:, nt : nt + 1, :],
                    )

            # ================= schedule =================
            # prologue: batch 0's qk and v
            for jt in range(JQK):
                qk_chain(0, jt)
            for nt in range(NT):
                for c0, cw in ((0, 384), (384, 384)):
                    v_chain(0, nt, c0, cw)

            for b in range(BL):
                oT = [
                    mid.tile([P, N], F16, tag=f"oT{kt}", name=f"oT{b}_{kt}")
                    for kt in range(KT)
                ]
                pending = s_phase_pair(b, 0)
                for p in range(H // 2):
                    nxt = s_phase_pair(b, p + 1) if p + 1 < H // 2 else None
                    # next batch's qk/v chains are spread BETWEEN the two
                    # softmax/attn@v phases so independent PE work sits
                    # exactly in the windows where attn@v waits on the
                    # exp->mask chain
                    if b + 1 < BL:
                        qk_chain(b + 1, 2 * p)
                    softmax_av_phase(b, 2 * p, pending[0], oT)
                    if b + 1 < BL:
                        qk_chain(b + 1, 2 * p + 1)
                    softmax_av_phase(b, 2 * p + 1, pending[1], oT)
                    if b + 1 < BL:
                        v_chain(b + 1, p // 2, (p % 2) * 384, 384)
                    pending = nxt
                proj_phase(b, oT)

    nc.compile()
    return nc


def _get_nc():
    if "nc" not in _CACHE:
        _CACHE["nc"] = _build_nc()
    return _CACHE["nc"]


def kernel(x, policy, qkv_w, qkv_b, proj_w, proj_b):
    from concourse.bass_utils import run_bass_kernel_spmd

    nc = _get_nc()

    x = np.asarray(x, dtype=np.float32)
    policy = np.asarray(policy, dtype=np.float32)
    qkv_w = np.asarray(qkv_w, dtype=np.float32)
    qkv_b = np.asarray(qkv_b, dtype=np.float32)
    proj_w = np.asarray(proj_w, dtype=np.float32)
    proj_b = np.asarray(proj_b, dtype=np.float32)

    xT = np.ascontiguousarray(
        x.transpose(0, 2, 1).reshape(B, KT, P, N).transpose(0, 2, 1, 3)
    ).astype(np.float16)  # [B, P, KT, N]
    pol = np.ascontiguousarray(
        policy.reshape(B, N).reshape(B, NT, P).transpose(0, 2, 1)
    )  # [B, P, NT]

    def to_sbuf_layout(w):  # [C, J] -> [P, KT, J]
        return np.ascontiguousarray(w.reshape(KT, P, -1).transpose(1, 0, 2))

    wqkT = to_sbuf_layout(qkv_w[: 2 * C].T.astype(np.float16))
    wpT = to_sbuf_layout(proj_w.T.astype(np.float16))
    bqk = np.ascontiguousarray(qkv_b[: 2 * C].reshape(JQK, P).T)  # [P, 12]
    wvT = to_sbuf_layout(qkv_w[2 * C :].T.astype(np.float16))
    bv = np.ascontiguousarray(qkv_b[2 * C :])

    in_maps = []
    for c in range(NCORES):
        s = slice(c * BL, (c + 1) * BL)
        in_maps.append({
            "xT": xT[s], "pol": pol[s],
            "wqkT": wqkT, "wvT": wvT, "bqk": bqk, "bv": bv,
            "wpT": wpT, "bp": proj_b,
        })

    res = run_bass_kernel_spmd(nc, in_maps, core_ids=list(range(NCORES)))
    _CACHE["last_results"] = res
    out = np.concatenate(
        [res.results[c]["out"] for c in range(NCORES)], axis=0
    ).astype(np.float32)
    return out
